# revision 9
# baseline (speedup 1.0000x reference)
"""Causal self-attention (B=2, L=2048, D=1024, H=16) on 8 Trainium2 cores.

Sharding: data-parallel over batch (2) x tensor-parallel over head groups
(4 groups of 4 heads) = 8 cores. Each core computes qkv projection, causal
flash-style attention and a partial out-projection for its 4 heads; the
out_proj partials are reduced on the host (row-parallel weight split), which
is the final "all-reduce" of the hint done during unsharding.

Device algorithm (per core), all matmuls in fp32r (full-rate fp32):
  - inputs pre-transposed on host: xT (D, L) so the contraction dim is the
    partition dim everywhere.
  - qkv: q^T, k^T (head-dim-major, 2 heads packed per 128 partitions),
    v (L-major). k^T is stored zero-padded per head (head h occupies
    partitions (h%2)*64..) so score matmuls contract over the full 128
    partitions while staying head-separated (avoids base-partition-64
    matmul operands, which fail on HW with fp32r).
  - scores computed transposed: S^T[s, l] = k^T.T @ q^T, so softmax's sum
    runs over the partition dim and is computed for free by appending a
    ones-column to v in the P^T @ v matmul (row 64 of the psum = denom).
  - causal masking: block-skip fully-masked tiles, narrow partial tiles to
    their valid column range, and apply a 128x128 triangular mask multiply
    only on the diagonal strip.
  - normalization (x 1/denom) is deferred to after the @v matmul: the recip
    row is partition-broadcast on GpSimd and applied in one DVE multiply.

Schedule: emission interleaves stage A block lb+1 and out_proj of block
j-1 into attention block j's head-pair chains so the exp-bound (ACT)
attention stretches are filled with independent PE work.
"""

import os
import sys

sys.path.insert(0, "/opt/trn_rl_repo")

import numpy as np

B, L, D, H, HD = 2, 2048, 1024, 16, 64
SCALE = HD ** -0.5
NCORES = 8
HL = 4            # heads per core
DL = HL * HD      # 256 local head dims
LB = 512          # L block width (psum bank)
NLB = L // LB     # 4
NST = L // 128    # 16 s-tiles
NCT = D // 128    # 8 contraction tiles

_prog_cache = {}


def _build_program():
    import concourse.bass as bass
    import concourse.tile as tile
    from concourse import bacc, mybir
    from concourse.bass import _add_dep_helper
    from concourse.masks import make_upper_triangular

    F32R, F32 = mybir.dt.float32r, mybir.dt.float32
    AF = mybir.ActivationFunctionType

    nc = bacc.Bacc("TRN2", target_bir_lowering=False, debug=False)
    xT = nc.dram_tensor("xT", [D, L], F32, kind="ExternalInput").ap()
    w = nc.dram_tensor("w", [D, 3 * DL], F32, kind="ExternalInput").ap()
    bqk = nc.dram_tensor("bqk", [4, 128], F32, kind="ExternalInput").ap()
    bv = nc.dram_tensor("bv", [1, DL], F32, kind="ExternalInput").ap()
    wo = nc.dram_tensor("wo", [DL, D], F32, kind="ExternalInput").ap()
    zz = nc.dram_tensor("zz", [L], F32, kind="ExternalInput").ap()
    out_p = nc.dram_tensor("out_p", [L, D], F32, kind="ExternalOutput").ap()
    k_out = nc.dram_tensor("k_out", [HL, HD, L], F32, kind="ExternalOutput").ap()
    v_out = nc.dram_tensor("v_out", [L, DL], F32, kind="ExternalOutput").ap()

    xT_r = xT.rearrange("(t p) l -> p t l", p=128).bitcast(F32R)
    w_r = w.rearrange("(t p) n -> p t n", p=128).bitcast(F32R)

    with tile.TileContext(nc) as tc:
        with (
            tc.tile_pool(name="const", bufs=1) as cpool,
            tc.tile_pool(name="xw", bufs=1) as xw,
            tc.tile_pool(name="ptp", bufs=6) as ptp,
            tc.tile_pool(name="rp", bufs=2) as rp,
            tc.tile_pool(name="outp", bufs=3) as outp,
            tc.tile_pool(name="psS", bufs=4, space="PSUM") as psS,
            tc.tile_pool(name="psO", bufs=2, space="PSUM") as psO,
            tc.tile_pool(name="psC", bufs=2, space="PSUM") as psC,
        ):
            q_t = cpool.tile([128, 2, L], F32R)          # q^T, 2 heads per pair tile
            k_t = cpool.tile([128, HL, L], F32R)         # k^T zero-padded per head
            v_aug = cpool.tile([128, NST, HL * 65], F32R)  # v s-major + ones col per head
            attn_t = cpool.tile([128, 2, L], F32R)       # attn_out^T (d_local, L)
            wo_t = cpool.tile([128, 2, D], F32R)
            bqk_sb = cpool.tile([128, 4], F32)
            bv_sb = cpool.tile([1, DL], F32R)
            ones32 = cpool.tile([128, 128], F32)
            onesr = cpool.tile([1, 128], F32R)
            tri32 = cpool.tile([128, 128], F32)
            tri = cpool.tile([128, 128], F32R)
            xT_t = xw.tile([128, NCT, L], F32R)
            w_t = xw.tile([128, NCT, 3 * DL], F32R)

            nc.vector.memset(ones32[:], 1.0)
            nc.scalar.copy(out=onesr[:], in_=ones32[0:1, :])
            make_upper_triangular(nc, tri32[:], val=1.0, diag=True)
            nc.scalar.copy(out=tri[:], in_=tri32[:])
            nc.sync.dma_start(
                out=bqk_sb[:],
                in_=bass.AP(tensor=bqk.tensor, offset=bqk.offset,
                            ap=[[1, 128], [128, 4]]),
            )
            nc.sync.dma_start(out=bv_sb[:], in_=bv[:].bitcast(F32R))
            nc.sync.dma_start(out=wo_t[:], in_=wo.rearrange("(t p) n -> p t n", p=128).bitcast(F32R))
            # zero-fill the padded halves of k_t (head h: inactive 64 partitions)
            for h in range(HL):
                p0 = 64 if h % 2 == 0 else 0
                nc.sync.dma_start(
                    out=k_t[p0:p0 + 64, h, :],
                    in_=bass.AP(tensor=zz.tensor, offset=zz.offset,
                                ap=[[0, 64], [1, L]]).bitcast(F32R),
                )
            # ones columns of v_aug (col 64 of each head's 65-wide group)
            nc.scalar.copy(
                out=v_aug.rearrange("p s (h e) -> p s h e", e=65)[:, :, :, 64:65],
                in_=ones32[:, 0:64].rearrange("p (s h) -> p s h", s=NST),
            )

            # x/w input loads: split per contraction tile and chained so they
            # arrive in order (stage A consumes c-tiles sequentially)
            prev = None
            for ct in range(NCT):
                dw = nc.sync.dma_start(out=w_t[:, ct, :], in_=w_r[:, ct, :])
                if prev is not None:
                    _add_dep_helper(dw.ins, prev.ins, sync=True,
                                    reason="serialize input loads")
                dx = nc.sync.dma_start(out=xT_t[:, ct, :], in_=xT_r[:, ct, :])
                _add_dep_helper(dx.ins, dw.ins, sync=True,
                                reason="serialize input loads")
                prev = dx

            # ---------------- emission helpers ----------------
            def stage_a_piece(kind, g, lb):
                """kind: 'q' | 'k' (pair g, L-block lb) or 'v' (s-tile g)."""
                if kind in ("q", "k"):
                    qk = 0 if kind == "q" else 1
                    ps = psC.tile([128, LB], F32, tag="pc")
                    for ct in range(NCT):
                        nc.tensor.matmul(
                            ps[:],
                            w_t[:, ct, qk * 256 + g * 128: qk * 256 + (g + 1) * 128],
                            xT_t[:, ct, lb * LB:(lb + 1) * LB],
                            start=(ct == 0),
                            stop=(ct == NCT - 1),
                        )
                    bcol = qk * 2 + g
                    if kind == "q":
                        dst0 = q_t[0:64, g, lb * LB:(lb + 1) * LB]
                        dst1 = q_t[64:128, g, lb * LB:(lb + 1) * LB]
                    else:
                        dst0 = k_t[0:64, 2 * g, lb * LB:(lb + 1) * LB]
                        dst1 = k_t[64:128, 2 * g + 1, lb * LB:(lb + 1) * LB]
                    nc.vector.tensor_scalar_add(dst0, ps[0:64, :], bqk_sb[0:64, bcol:bcol + 1])
                    nc.vector.tensor_scalar_add(dst1, ps[64:128, :], bqk_sb[64:128, bcol:bcol + 1])
                    if kind == "k":
                        # k output (head-dim-major slice of this L-block)
                        for h in (2 * g, 2 * g + 1):
                            p0 = (h % 2) * 64
                            nc.sync.dma_start(
                                out=k_out[h][:, lb * LB:(lb + 1) * LB],
                                in_=k_t[p0:p0 + 64, h, lb * LB:(lb + 1) * LB].bitcast(F32),
                            )
                else:
                    st = g
                    psv = psC.tile([128, DL], F32, tag="pc")
                    for ct in range(NCT):
                        nc.tensor.matmul(
                            psv[:],
                            xT_t[:, ct, st * 128:(st + 1) * 128],
                            w_t[:, ct, 512:768],
                            start=(ct == 0),
                            stop=False,
                        )
                    nc.tensor.matmul(
                        psv[:], onesr[:, 0:128], bv_sb[:], start=False, stop=True
                    )
                    nc.vector.tensor_copy(
                        v_aug.rearrange("p s (h e) -> p s h e", e=65)[:, st, :, 0:64],
                        psv.rearrange("p (h e) -> p h e", e=64),
                    )
                    nc.sync.dma_start(
                        out=v_out[st * 128:(st + 1) * 128, :],
                        in_=v_aug.rearrange("p s (h e) -> p s h e", e=65)[:, st, :, 0:64].bitcast(F32),
                    )

            def out_proj_unit(li, nb):
                psf = psC.tile([128, LB], F32, tag="pc")
                for kt in range(2):
                    nc.tensor.matmul(
                        psf[:],
                        attn_t[:, kt, li * 128:(li + 1) * 128],
                        wo_t[:, kt, nb * LB:(nb + 1) * LB],
                        start=(kt == 0),
                        stop=(kt == 1),
                    )
                ot = outp.tile([128, LB], F32, tag="ot")
                nc.vector.tensor_copy(ot[:], psf[:])
                nc.sync.dma_start(
                    out=out_p[li * 128:(li + 1) * 128, nb * LB:(nb + 1) * LB],
                    in_=ot[:],
                )

            def finalize_head(h, j, pso):
                g = h // 2
                r = rp.tile([1, LB], F32, tag="r")
                nc.vector.reciprocal(out=r[:], in_=pso[64:65, :])
                rb = rp.tile([64, LB], F32, tag="rb")
                nc.gpsimd.partition_broadcast(rb[:], r[:])
                p0 = (h % 2) * 64
                nc.vector.tensor_mul(
                    attn_t[p0:p0 + 64, g, j * LB:(j + 1) * LB],
                    pso[0:64, :],
                    rb[:],
                )

            # fillers: independent PE work inserted into ACT-bound chains.
            # Each entry is (block_tag, fn); block_tag marks stage-A work for
            # a given block so it can be force-drained before that block's
            # chains consume it.
            fillers = []

            def emit_filler():
                if fillers:
                    fillers.pop(0)[1]()

            def drain_fillers_for(block):
                rest = []
                for tag, fn in fillers:
                    if tag == block:
                        fn()
                    else:
                        rest.append((tag, fn))
                fillers[:] = rest

            # ---------------- main schedule ----------------
            # stage A for block 0 (cannot overlap with anything useful)
            for kind, g in (("k", 0), ("q", 0), ("v", 0), ("v", 1), ("v", 2), ("v", 3),
                            ("k", 1), ("q", 1)):
                stage_a_piece(kind, g, 0)

            for j in range(NLB):
                # anything this block's chains depend on must be emitted now
                drain_fillers_for(j)
                # queue next stage-A block / previous out_proj as fillers
                if j + 1 < NLB:
                    for kind, g in (("k", 0), ("q", 0), ("k", 1), ("q", 1)):
                        fillers.append((j + 1, lambda k=kind, gg=g, lb=j + 1: stage_a_piece(k, gg, lb)))
                    for st in range(4 * (j + 1), 4 * (j + 2)):
                        fillers.append((j + 1, lambda s=st: stage_a_piece("v", s, 0)))
                if j > 0:
                    for li in range(4 * (j - 1), 4 * j):
                        for nb in range(2):
                            fillers.append((None, lambda a=li, b=nb: out_proj_unit(a, b)))

                n_st = 4 * (j + 1)
                for pair in range(2):
                    ha, hb = 2 * pair, 2 * pair + 1
                    pso_a = psO.tile([65, LB], F32, tag="po")
                    pso_b = psO.tile([65, LB], F32, tag="po")
                    for st in range(n_st):
                        d_off = st * 128 - j * LB
                        col0 = max(d_off, 0)
                        for h, pso in ((ha, pso_a), (hb, pso_b)):
                            pss = psS.tile([128, LB], F32, tag="ps")
                            nc.tensor.matmul(
                                pss[:, col0:LB],
                                k_t[:, h, st * 128:(st + 1) * 128],
                                q_t[:, h // 2, j * LB + col0:(j + 1) * LB],
                                start=True, stop=True,
                            )
                            pt = ptp.tile([128, LB], F32R, tag="pt")
                            nc.scalar.activation(
                                out=pt[:, col0:LB], in_=pss[:, col0:LB],
                                func=AF.Exp, scale=SCALE,
                            )
                            if d_off >= 0:
                                nc.vector.tensor_mul(
                                    pt[:, d_off:d_off + 128],
                                    pt[:, d_off:d_off + 128],
                                    tri[:],
                                )
                            nc.tensor.matmul(
                                pso[:, col0:LB],
                                v_aug[:, st, h * 65:(h + 1) * 65],
                                pt[:, col0:LB],
                                start=(st == 0),
                                stop=(st == n_st - 1),
                            )
                        if st % 2 == 1:
                            emit_filler()
                    finalize_head(ha, j, pso_a)
                    finalize_head(hb, j, pso_b)
                    emit_filler()

            # drain remaining fillers + final block's out_proj
            while fillers:
                emit_filler()
            for li in range(4 * (NLB - 1), 4 * NLB):
                for nb in range(2):
                    out_proj_unit(li, nb)

    nc.compile()
    return nc


def _get_program():
    if "nc" not in _prog_cache:
        _prog_cache["nc"] = _build_program()
    return _prog_cache["nc"]


def _shard_inputs(x, w_qkv, b_qkv, w_out):
    """Build the 8 per-core input maps."""
    zeros = np.zeros(L, np.float32)
    in_maps = []
    for core in range(NCORES):
        b = core // 4
        hg = core % 4
        heads = list(range(hg * HL, (hg + 1) * HL))
        qcols = np.concatenate([w_qkv[:, h * HD:(h + 1) * HD] for h in heads], axis=1)
        kcols = np.concatenate([w_qkv[:, D + h * HD:D + (h + 1) * HD] for h in heads], axis=1)
        vcols = np.concatenate([w_qkv[:, 2 * D + h * HD:2 * D + (h + 1) * HD] for h in heads], axis=1)
        w_local = np.ascontiguousarray(np.concatenate([qcols, kcols, vcols], axis=1))
        bq = np.concatenate([b_qkv[h * HD:(h + 1) * HD] for h in heads])
        bk = np.concatenate([b_qkv[D + h * HD:D + (h + 1) * HD] for h in heads])
        bvv = np.concatenate([b_qkv[2 * D + h * HD:2 * D + (h + 1) * HD] for h in heads])
        bqk_rows = np.stack([bq[0:128], bq[128:256], bk[0:128], bk[128:256]])
        wo_local = np.ascontiguousarray(
            np.concatenate([w_out[h * HD:(h + 1) * HD, :] for h in heads], axis=0)
        )
        in_maps.append({
            "xT": np.ascontiguousarray(x[b].T),
            "w": w_local,
            "bqk": np.ascontiguousarray(bqk_rows),
            "bv": np.ascontiguousarray(bvv[None, :]),
            "wo": wo_local,
            "zz": zeros,
        })
    return in_maps


def kernel(**inputs):
    x = np.asarray(inputs["x"], np.float32)
    w_qkv = np.asarray(inputs["w_qkv"], np.float32)
    b_qkv = np.asarray(inputs["b_qkv"], np.float32)
    w_out = np.asarray(inputs["w_out"], np.float32)
    b_out = np.asarray(inputs["b_out"], np.float32)

    nc = _get_program()
    in_maps = _shard_inputs(x, w_qkv, b_qkv, w_out)

    from concourse.bass_utils import run_bass_kernel_spmd

    res = run_bass_kernel_spmd(nc, in_maps, core_ids=list(range(NCORES)))

    out = np.zeros((B, L, D), np.float32)
    k = np.empty((B, H, L, HD), np.float32)
    v = np.empty((B, H, L, HD), np.float32)
    for core in range(NCORES):
        b = core // 4
        hg = core % 4
        r = res.results[core]
        out[b] += r["out_p"]
        for i in range(HL):
            h = hg * HL + i
            k[b, h] = r["k_out"][i].T
            v[b, h] = r["v_out"][:, i * HD:(i + 1) * HD]
    out += b_out[None, None, :]
    return out, k, v


# revision 13
# speedup vs baseline: 1.2238x; 1.2238x over previous
"""Causal self-attention (B=2, L=2048, D=1024, H=16) on 8 Trainium2 cores.

Sharding: data-parallel over batch (2) x tensor-parallel over head groups
(4 groups of 4 heads) = 8 cores. Each core computes qkv projection, causal
flash-style attention and a partial out-projection for its 4 heads; the
out_proj partials are reduced on the host (row-parallel weight split), which
is the final "all-reduce" of the hint done during unsharding.

Device algorithm (per core), all matmuls in fp32r (full-rate fp32):
  - inputs pre-transposed on host: xT (D, L) so the contraction dim is the
    partition dim everywhere.
  - qkv: q^T, k^T (head-dim-major, 2 heads packed per 128 partitions),
    v (L-major). k^T is stored zero-padded per head (head h occupies
    partitions (h%2)*64..) so score matmuls contract over the full 128
    partitions while staying head-separated (avoids base-partition-64
    matmul operands, which fail on HW with fp32r).
  - scores computed transposed: S^T[s, l] = k^T.T @ q^T, so softmax's sum
    runs over the partition dim and is computed for free by appending a
    ones-column to v in the P^T @ v matmul (row 64 of the psum = denom).
  - causal masking: block-skip fully-masked tiles, narrow partial tiles to
    their valid column range, and apply a 128x128 triangular mask multiply
    only on the diagonal strip.
  - normalization (x 1/denom) is deferred to after the @v matmul: the recip
    row is partition-broadcast on GpSimd and applied in one DVE multiply.

Schedule: emission interleaves stage A block lb+1 and out_proj of block
j-1 into attention block j's head-pair chains so the exp-bound (ACT)
attention stretches are filled with independent PE work.
"""

import os
import sys

sys.path.insert(0, "/opt/trn_rl_repo")

import numpy as np

B, L, D, H, HD = 2, 2048, 1024, 16, 64
SCALE = HD ** -0.5
NCORES = 8
HL = 4            # heads per core
DL = HL * HD      # 256 local head dims
LB = 512          # L block width (psum bank)
NLB = L // LB     # 4
NST = L // 128    # 16 s-tiles
NCT = D // 128    # 8 contraction tiles

_prog_cache = {}


def _build_program():
    import concourse.bass as bass
    import concourse.tile as tile
    from concourse import bacc, mybir
    from concourse.masks import make_upper_triangular

    F32R, F32 = mybir.dt.float32r, mybir.dt.float32
    AF = mybir.ActivationFunctionType

    nc = bacc.Bacc("TRN2", target_bir_lowering=False, debug=False)
    xT = nc.dram_tensor("xT", [D, L], F32, kind="ExternalInput").ap()
    w = nc.dram_tensor("w", [D, 3 * DL], F32, kind="ExternalInput").ap()
    bqk = nc.dram_tensor("bqk", [4, 128], F32, kind="ExternalInput").ap()
    bv = nc.dram_tensor("bv", [1, DL], F32, kind="ExternalInput").ap()
    wo = nc.dram_tensor("wo", [DL, D], F32, kind="ExternalInput").ap()
    zz = nc.dram_tensor("zz", [L], F32, kind="ExternalInput").ap()
    out_p = nc.dram_tensor("out_p", [L, D], F32, kind="ExternalOutput").ap()
    k_out = nc.dram_tensor("k_out", [HL, HD, L], F32, kind="ExternalOutput").ap()
    v_out = nc.dram_tensor("v_out", [L, DL], F32, kind="ExternalOutput").ap()

    xT_r = xT.rearrange("(t p) l -> p t l", p=128).bitcast(F32R)
    w_r = w.rearrange("(t p) n -> p t n", p=128).bitcast(F32R)

    with tile.TileContext(nc) as tc:
        with (
            tc.tile_pool(name="const", bufs=1) as cpool,
            tc.tile_pool(name="xw", bufs=1) as xw,
            tc.tile_pool(name="ptp", bufs=6) as ptp,
            tc.tile_pool(name="rp", bufs=2) as rp,
            tc.tile_pool(name="outp", bufs=3) as outp,
            tc.tile_pool(name="psS", bufs=4, space="PSUM") as psS,
            tc.tile_pool(name="psO", bufs=2, space="PSUM") as psO,
            tc.tile_pool(name="psC", bufs=2, space="PSUM") as psC,
        ):
            q_t = cpool.tile([128, 2, L], F32R)          # q^T, 2 heads per pair tile
            k_t = cpool.tile([128, HL, L], F32R)         # k^T zero-padded per head
            v_aug = cpool.tile([128, NST, HL * 65], F32R)  # v s-major + ones col per head
            attn_t = cpool.tile([128, 2, L], F32R)       # attn_out^T (d_local, L)
            wo_t = cpool.tile([128, 2, D], F32R)
            bqk_sb = cpool.tile([128, 4], F32)
            bv_sb = cpool.tile([1, DL], F32R)
            ones32 = cpool.tile([128, 128], F32)
            onesr = cpool.tile([1, 128], F32R)
            tri32 = cpool.tile([128, 128], F32)
            tri = cpool.tile([128, 128], F32R)
            xT_t = xw.tile([128, NCT, L], F32R)
            w_t = xw.tile([128, NCT, 3 * DL], F32R)

            nc.vector.memset(ones32[:], 1.0)
            nc.scalar.copy(out=onesr[:], in_=ones32[0:1, :])
            make_upper_triangular(nc, tri32[:], val=1.0, diag=True)
            nc.scalar.copy(out=tri[:], in_=tri32[:])
            nc.sync.dma_start(
                out=bqk_sb[:],
                in_=bass.AP(tensor=bqk.tensor, offset=bqk.offset,
                            ap=[[1, 128], [128, 4]]),
            )
            nc.sync.dma_start(out=bv_sb[:], in_=bv[:].bitcast(F32R))

            # x/w input loads first (everything else on the DMA queues can
            # wait); split per contraction tile so stage A starts on c-tile 0
            for ct in range(NCT):
                nc.sync.dma_start(out=w_t[:, ct, :], in_=w_r[:, ct, :])
                nc.sync.dma_start(out=xT_t[:, ct, :], in_=xT_r[:, ct, :])

            # zero-fill the padded halves of k_t (head h: inactive 64 partitions)
            for h in range(HL):
                p0 = 64 if h % 2 == 0 else 0
                nc.sync.dma_start(
                    out=k_t[p0:p0 + 64, h, :],
                    in_=bass.AP(tensor=zz.tensor, offset=zz.offset,
                                ap=[[0, 64], [1, L]]).bitcast(F32R),
                )
            # ones columns of v_aug (col 64 of each head's 65-wide group)
            nc.scalar.copy(
                out=v_aug.rearrange("p s (h e) -> p s h e", e=65)[:, :, :, 64:65],
                in_=ones32[:, 0:64].rearrange("p (s h) -> p s h", s=NST),
            )
            nc.sync.dma_start(out=wo_t[:], in_=wo.rearrange("(t p) n -> p t n", p=128).bitcast(F32R))

            # ---------------- emission helpers ----------------
            def stage_a_piece(kind, g, lb):
                """kind: 'q' | 'k' (pair g, L-block lb) or 'v' (s-tile g)."""
                if kind in ("q", "k"):
                    qk = 0 if kind == "q" else 1
                    ps = psC.tile([128, LB], F32, tag="pc")
                    for ct in range(NCT):
                        nc.tensor.matmul(
                            ps[:],
                            w_t[:, ct, qk * 256 + g * 128: qk * 256 + (g + 1) * 128],
                            xT_t[:, ct, lb * LB:(lb + 1) * LB],
                            start=(ct == 0),
                            stop=(ct == NCT - 1),
                        )
                    bcol = qk * 2 + g
                    if kind == "q":
                        dst0 = q_t[0:64, g, lb * LB:(lb + 1) * LB]
                        dst1 = q_t[64:128, g, lb * LB:(lb + 1) * LB]
                    else:
                        dst0 = k_t[0:64, 2 * g, lb * LB:(lb + 1) * LB]
                        dst1 = k_t[64:128, 2 * g + 1, lb * LB:(lb + 1) * LB]
                    nc.vector.tensor_scalar_add(dst0, ps[0:64, :], bqk_sb[0:64, bcol:bcol + 1])
                    nc.vector.tensor_scalar_add(dst1, ps[64:128, :], bqk_sb[64:128, bcol:bcol + 1])
                    if kind == "k":
                        # k output (head-dim-major slice of this L-block)
                        for h in (2 * g, 2 * g + 1):
                            p0 = (h % 2) * 64
                            nc.sync.dma_start(
                                out=k_out[h][:, lb * LB:(lb + 1) * LB],
                                in_=k_t[p0:p0 + 64, h, lb * LB:(lb + 1) * LB].bitcast(F32),
                            )
                else:
                    st = g
                    psv = psC.tile([128, DL], F32, tag="pc")
                    for ct in range(NCT):
                        nc.tensor.matmul(
                            psv[:],
                            xT_t[:, ct, st * 128:(st + 1) * 128],
                            w_t[:, ct, 512:768],
                            start=(ct == 0),
                            stop=False,
                        )
                    nc.tensor.matmul(
                        psv[:], onesr[:, 0:128], bv_sb[:], start=False, stop=True
                    )
                    nc.vector.tensor_copy(
                        v_aug.rearrange("p s (h e) -> p s h e", e=65)[:, st, :, 0:64],
                        psv.rearrange("p (h e) -> p h e", e=64),
                    )
                    nc.sync.dma_start(
                        out=v_out[st * 128:(st + 1) * 128, :],
                        in_=v_aug.rearrange("p s (h e) -> p s h e", e=65)[:, st, :, 0:64].bitcast(F32),
                    )

            def out_proj_unit(li, nb):
                psf = psC.tile([128, LB], F32, tag="pc")
                for kt in range(2):
                    nc.tensor.matmul(
                        psf[:],
                        attn_t[:, kt, li * 128:(li + 1) * 128],
                        wo_t[:, kt, nb * LB:(nb + 1) * LB],
                        start=(kt == 0),
                        stop=(kt == 1),
                    )
                ot = outp.tile([128, LB], F32, tag="ot")
                nc.vector.tensor_copy(ot[:], psf[:])
                nc.sync.dma_start(
                    out=out_p[li * 128:(li + 1) * 128, nb * LB:(nb + 1) * LB],
                    in_=ot[:],
                )

            def finalize_head(h, j, pso):
                g = h // 2
                r = rp.tile([1, LB], F32, tag="r")
                nc.vector.reciprocal(out=r[:], in_=pso[64:65, :])
                rb = rp.tile([64, LB], F32, tag="rb")
                nc.gpsimd.partition_broadcast(rb[:], r[:])
                p0 = (h % 2) * 64
                nc.vector.tensor_mul(
                    attn_t[p0:p0 + 64, g, j * LB:(j + 1) * LB],
                    pso[0:64, :],
                    rb[:],
                )

            # fillers: independent PE work inserted into ACT-bound chains.
            # Each entry is (block_tag, fn); block_tag marks stage-A work for
            # a given block so it can be force-drained before that block's
            # chains consume it.
            fillers = []

            def emit_filler():
                if fillers:
                    fillers.pop(0)[1]()

            def drain_fillers_for(block):
                rest = []
                for tag, fn in fillers:
                    if tag == block:
                        fn()
                    else:
                        rest.append((tag, fn))
                fillers[:] = rest

            # ---------------- main schedule ----------------
            # stage A critical pieces for block 0 pair 0; the g1 pieces are
            # emitted between pair 0 and pair 1 below
            for kind, g in (("k", 0), ("q", 0), ("v", 0), ("v", 1), ("v", 2), ("v", 3)):
                stage_a_piece(kind, g, 0)

            for j in range(NLB):
                # anything this block's chains depend on must be emitted now
                drain_fillers_for(j)
                # queue next stage-A block / previous out_proj as fillers
                if j + 1 < NLB:
                    for kind, g in (("k", 0), ("q", 0), ("k", 1), ("q", 1)):
                        fillers.append((j + 1, lambda k=kind, gg=g, lb=j + 1: stage_a_piece(k, gg, lb)))
                    for st in range(4 * (j + 1), 4 * (j + 2)):
                        fillers.append((j + 1, lambda s=st: stage_a_piece("v", s, 0)))
                if j > 0:
                    for li in range(4 * (j - 1), 4 * j):
                        for nb in range(2):
                            fillers.append((None, lambda a=li, b=nb: out_proj_unit(a, b)))

                n_st = 4 * (j + 1)
                for pair in range(2):
                    if j == 0 and pair == 1:
                        stage_a_piece("k", 1, 0)
                        stage_a_piece("q", 1, 0)
                    ha, hb = 2 * pair, 2 * pair + 1
                    pso_a = psO.tile([65, LB], F32, tag="po")
                    pso_b = psO.tile([65, LB], F32, tag="po")
                    for st in range(n_st):
                        d_off = st * 128 - j * LB
                        col0 = max(d_off, 0)
                        for h, pso in ((ha, pso_a), (hb, pso_b)):
                            pss = psS.tile([128, LB], F32, tag="ps")
                            nc.tensor.matmul(
                                pss[:, col0:LB],
                                k_t[:, h, st * 128:(st + 1) * 128],
                                q_t[:, h // 2, j * LB + col0:(j + 1) * LB],
                                start=True, stop=True,
                            )
                            pt = ptp.tile([128, LB], F32R, tag="pt")
                            nc.scalar.activation(
                                out=pt[:, col0:LB], in_=pss[:, col0:LB],
                                func=AF.Exp, scale=SCALE,
                            )
                            if d_off >= 0:
                                nc.vector.tensor_mul(
                                    pt[:, d_off:d_off + 128],
                                    pt[:, d_off:d_off + 128],
                                    tri[:],
                                )
                            nc.tensor.matmul(
                                pso[:, col0:LB],
                                v_aug[:, st, h * 65:(h + 1) * 65],
                                pt[:, col0:LB],
                                start=(st == 0),
                                stop=(st == n_st - 1),
                            )
                        if st % 2 == 1:
                            emit_filler()
                    finalize_head(ha, j, pso_a)
                    finalize_head(hb, j, pso_b)
                    emit_filler()

            # drain remaining fillers + final block's out_proj
            while fillers:
                emit_filler()
            for li in range(4 * (NLB - 1), 4 * NLB):
                for nb in range(2):
                    out_proj_unit(li, nb)

    nc.compile()
    return nc


def _get_program():
    if "nc" not in _prog_cache:
        _prog_cache["nc"] = _build_program()
    return _prog_cache["nc"]


def _shard_inputs(x, w_qkv, b_qkv, w_out):
    """Build the 8 per-core input maps."""
    zeros = np.zeros(L, np.float32)
    in_maps = []
    for core in range(NCORES):
        b = core // 4
        hg = core % 4
        heads = list(range(hg * HL, (hg + 1) * HL))
        qcols = np.concatenate([w_qkv[:, h * HD:(h + 1) * HD] for h in heads], axis=1)
        kcols = np.concatenate([w_qkv[:, D + h * HD:D + (h + 1) * HD] for h in heads], axis=1)
        vcols = np.concatenate([w_qkv[:, 2 * D + h * HD:2 * D + (h + 1) * HD] for h in heads], axis=1)
        w_local = np.ascontiguousarray(np.concatenate([qcols, kcols, vcols], axis=1))
        bq = np.concatenate([b_qkv[h * HD:(h + 1) * HD] for h in heads])
        bk = np.concatenate([b_qkv[D + h * HD:D + (h + 1) * HD] for h in heads])
        bvv = np.concatenate([b_qkv[2 * D + h * HD:2 * D + (h + 1) * HD] for h in heads])
        bqk_rows = np.stack([bq[0:128], bq[128:256], bk[0:128], bk[128:256]])
        wo_local = np.ascontiguousarray(
            np.concatenate([w_out[h * HD:(h + 1) * HD, :] for h in heads], axis=0)
        )
        in_maps.append({
            "xT": np.ascontiguousarray(x[b].T),
            "w": w_local,
            "bqk": np.ascontiguousarray(bqk_rows),
            "bv": np.ascontiguousarray(bvv[None, :]),
            "wo": wo_local,
            "zz": zeros,
        })
    return in_maps


def kernel(**inputs):
    x = np.asarray(inputs["x"], np.float32)
    w_qkv = np.asarray(inputs["w_qkv"], np.float32)
    b_qkv = np.asarray(inputs["b_qkv"], np.float32)
    w_out = np.asarray(inputs["w_out"], np.float32)
    b_out = np.asarray(inputs["b_out"], np.float32)

    nc = _get_program()
    in_maps = _shard_inputs(x, w_qkv, b_qkv, w_out)

    from concourse.bass_utils import run_bass_kernel_spmd

    res = run_bass_kernel_spmd(nc, in_maps, core_ids=list(range(NCORES)))

    out = np.zeros((B, L, D), np.float32)
    k = np.empty((B, H, L, HD), np.float32)
    v = np.empty((B, H, L, HD), np.float32)
    for core in range(NCORES):
        b = core // 4
        hg = core % 4
        r = res.results[core]
        out[b] += r["out_p"]
        for i in range(HL):
            h = hg * HL + i
            k[b, h] = r["k_out"][i].T
            v[b, h] = r["v_out"][:, i * HD:(i + 1) * HD]
    out += b_out[None, None, :]
    return out, k, v
